# revision 1
# baseline (speedup 1.0000x reference)
"""ArrowAttention distributed Trainium2 kernel (8 NeuronCores).

Sharding: 8-way sequence-parallel. Core i owns tokens [i*256,(i+1)*256) of
each batch (B=2,S=2048,E=1024,H=16,D=64,WIN=16). Weights replicated.

Host-side prep (make_shards): weights and the per-core hidden-state halo
are pre-transposed to feature-major layout and cast to bf16 -- pure data
layout / precision prep, zero module flops. On device, activations stay
feature-major; local banded attention runs q-major ([q, keys]) so the
softmax reduction is along the free axis (ACT Exp with accum_out), then
the normalized attention is PE-transposed for the ctx matmuls. CLS
attention is computed as per-core partials over the core's 256-token key
slice (all 16 heads) via a block-diagonal Qtilde, combined with one small
AllGather that overlaps the local-attention phase.

Biases are guaranteed zero by the problem spec (fill: zeros); they are
added on the host (exact).
"""
import sys
sys.path.insert(0, "/opt/trn_rl_repo")

import numpy as np
import concourse.bass as bass
import concourse.bacc as bacc
import concourse.mybir as mybir
import concourse.tile as tile
from concourse.bass_utils import run_bass_kernel_spmd

B, S, E, H, D, WIN = 2, 2048, 1024, 16, 64, 16
NC = 8
TPC = S // NC          # 256 query tokens per core per batch
HALO = TPC + 2 * WIN   # 288 halo tokens per batch
HPAD = 384             # halo padded to 3 partition tiles
NQT = TPC // 128       # 2 query tiles of 128 per batch
KW = 160               # key window per 128-query tile
ECH = E // 128         # 8 contraction chunks
W2 = ECH * 2 * H       # 256: packed cls-partial width
F32 = mybir.dt.float32
BF16 = mybir.dt.bfloat16
AF = mybir.ActivationFunctionType
SCALE = 1.0 / np.sqrt(D)
NEG = -30000.0         # additive mask for invalid keys


def build_kernel():
    nc = bacc.Bacc("TRN2", target_bir_lowering=False, debug=False, num_devices=NC)

    hT_ext = nc.dram_tensor("hT", [E, 2 * HPAD], BF16, kind="ExternalInput")
    h0T_ext = nc.dram_tensor("h0T", [128, ECH * B], BF16, kind="ExternalInput")
    w_ext = {name: nc.dram_tensor(name, [E, E], BF16, kind="ExternalInput")
             for name in ["wqT", "wkT", "wvT", "wqcT", "wkcT", "wvcT",
                          "woT", "wocT"]}
    mq_ext = nc.dram_tensor("mask_q", [128, NQT * KW], BF16, kind="ExternalInput")
    mcls_ext = nc.dram_tensor("mask_cls", [128, 2 * 2 * H], BF16, kind="ExternalInput")
    ehot_ext = nc.dram_tensor("ehot", [H, E], BF16, kind="ExternalInput")
    eye_ext = nc.dram_tensor("eye", [128, 128], BF16, kind="ExternalInput")
    out_ext = nc.dram_tensor("out", [2 * TPC + B, E], F32, kind="ExternalOutput")

    with tile.TileContext(nc) as tc:
        with (
            tc.tile_pool(name="const", bufs=1) as constp,
            tc.tile_pool(name="wT", bufs=4) as wTp,
            tc.tile_pool(name="acts", bufs=1) as actsp,
            tc.tile_pool(name="attn", bufs=6) as attnp,
            tc.tile_pool(name="cls", bufs=1) as clsp,
            tc.tile_pool(name="outst", bufs=2) as outstp,
            # PSUM: 8 banks. projps 2 + attnps 2 + ctxps 2 + tps 2.
            tc.tile_pool(name="psproj", bufs=2, space="PSUM") as psproj,
            tc.tile_pool(name="psattn", bufs=2, space="PSUM") as psattn,
            tc.tile_pool(name="psctx", bufs=2, space="PSUM") as psctxp,
            tc.tile_pool(name="pst", bufs=2, space="PSUM") as pstp,
            tc.tile_pool(name="dram", bufs=1, space="DRAM") as dramp,
        ):
            # ---------------- constants ----------------
            eye = constp.tile([128, 128], BF16, tag="eye")
            nc.gpsimd.dma_start(eye[:], eye_ext[:])
            ones_col = constp.tile([128, 1], BF16, tag="onesc")
            nc.gpsimd.memset(ones_col[:], 1.0)
            mask_q = constp.tile([128, NQT * KW], BF16, tag="mq")
            nc.gpsimd.dma_start(mask_q[:], mq_ext[:])
            mask_cls = constp.tile([128, 2 * 2 * H], BF16, tag="mcls")
            nc.gpsimd.dma_start(mask_cls[:], mcls_ext[:])
            ehot = constp.tile([H, E], BF16, tag="ehot")
            nc.gpsimd.dma_start(ehot[:], ehot_ext[:])
            h0T = constp.tile([128, ECH * B], BF16, tag="h0T")
            nc.sync.dma_start(h0T[:], h0T_ext[:])

            hT = [actsp.tile([128, 2 * HPAD], BF16, tag=f"hT{c}", name=f"hT{c}")
                  for c in range(ECH)]
            for c in range(ECH):
                nc.gpsimd.dma_start(hT[c][:], hT_ext[c * 128:(c + 1) * 128, :])

            def load_wT(name):
                wT = [wTp.tile([128, E], BF16, tag=f"wT{c}", name=f"{name}{c}")
                      for c in range(ECH)]
                for c in range(ECH):
                    nc.sync.dma_start(wT[c][:],
                                      w_ext[name][c * 128:(c + 1) * 128, :])
                return wT

            def own_cols(hc):
                return hc.rearrange("p (b t) -> p b t", b=2)[:, :, WIN:WIN + TPC]

            def proj_T(wT, rhs_of, n_t, scale=None, tagp=""):
                """Feature-major projection: ECH tiles [128 d_out, n_t]."""
                outT = [actsp.tile([128, n_t], BF16, tag=f"pT{tagp}{d}",
                                   name=f"pT{tagp}{d}") for d in range(ECH)]
                for d in range(ECH):
                    for n0 in range(0, n_t, 512):
                        nn = min(512, n_t - n0)
                        ps = psproj.tile([128, 512], F32, tag="projps",
                                         name=f"pp{tagp}{d}_{n0}")
                        for c in range(ECH):
                            nc.tensor.matmul(
                                ps[:, :nn], wT[c][:, d * 128:(d + 1) * 128],
                                rhs_of(wTc=hT[c], n0=n0, nn=nn),
                                start=(c == 0), stop=(c == ECH - 1))
                        if scale is None:
                            nc.scalar.copy(outT[d][:, n0:n0 + nn], ps[:, :nn])
                        else:
                            nc.scalar.activation(outT[d][:, n0:n0 + nn],
                                                 ps[:, :nn], AF.Copy,
                                                 scale=scale)
                return outT

            def proj_tok(wT, cols_of, n_t, tagp=""):
                """Token-major projection: n_t//128 tiles [128 t, E d_out]."""
                out = [actsp.tile([128, E], BF16, tag=f"v{tagp}{t}",
                                  name=f"v{tagp}{t}") for t in range(n_t // 128)]
                for t in range(n_t // 128):
                    for n0 in range(0, E, 512):
                        ps = psproj.tile([128, 512], F32, tag="projps",
                                         name=f"ppt{tagp}{t}_{n0}")
                        for c in range(ECH):
                            nc.tensor.matmul(
                                ps[:], cols_of(hT[c], t), wT[c][:, n0:n0 + 512],
                                start=(c == 0), stop=(c == ECH - 1))
                        nc.scalar.copy(out[t][:, n0:n0 + 512], ps[:])
                return out

            # ============ CLS phase (first: its AllGather overlaps) ============
            wkcT = load_wT("wkcT")
            kcT = proj_T(wkcT, lambda wTc, n0, nn: own_cols(wTc), 2 * TPC, tagp="kc")
            wqcT = load_wT("wqcT")
            qcT = clsp.tile([128, ECH * B], BF16, tag="qcT")
            for d in range(ECH):
                ps = psattn.tile([128, 512], F32, tag="attnps",
                                 name=f"qcps{d}")
                for c in range(ECH):
                    nc.tensor.matmul(ps[:, :B], wqcT[c][:, d * 128:(d + 1) * 128],
                                     h0T[:, c * B:(c + 1) * B],
                                     start=(c == 0), stop=(c == ECH - 1))
                nc.scalar.activation(qcT[:, d * B:(d + 1) * B], ps[:, :B],
                                     AF.Copy, scale=SCALE)
            qtl = clsp.tile([128, ECH * 2 * H], BF16, tag="qtl")
            nc.gpsimd.memset(qtl[:], 0.0)
            for g in range(ECH):
                for half in range(2):
                    h_ = 2 * g + half
                    for b in range(B):
                        nc.vector.tensor_copy(
                            qtl[half * 64:(half + 1) * 64,
                                g * 2 * H + 2 * h_ + b:g * 2 * H + 2 * h_ + b + 1],
                            qcT[half * 64:(half + 1) * 64, g * B + b:g * B + b + 1])

            wvcT = load_wT("wvcT")
            vc = proj_tok(
                wvcT,
                lambda hc, t: hc[:, (t // NQT) * HPAD + WIN + (t % NQT) * 128:
                                 (t // NQT) * HPAD + WIN + (t % NQT) * 128 + 128],
                2 * TPC, tagp="c")

            ctxu = clsp.tile([128, W2], F32, tag="ctxu")
            l_sb = clsp.tile([1, 2 * H], F32, tag="lsb")
            for jt in range(2 * NQT):
                b = jt // NQT
                ps = psattn.tile([128, 512], F32, tag="attnps",
                                 name=f"clssc{jt}")
                sc = ps[:, 0:2 * H]
                for c in range(ECH):
                    nc.tensor.matmul(sc, kcT[c][:, jt * 128:(jt + 1) * 128],
                                     qtl[:, c * 2 * H:(c + 1) * 2 * H],
                                     start=(c == 0), stop=(c == ECH - 1))
                exm = attnp.tile([128, 2 * H], BF16, tag="clsex",
                                 name=f"clsex{jt}")
                nc.scalar.activation(exm[:], sc, AF.Exp)
                nc.vector.tensor_mul(exm[:], exm[:],
                                     mask_cls[:, b * 2 * H:(b + 1) * 2 * H])
                psl = ps[0:1, 64:64 + 2 * H]
                nc.tensor.matmul(psl, ones_col[:], exm[:], start=True,
                                 stop=True)
                if jt == 0:
                    nc.vector.tensor_copy(l_sb[:], psl)
                else:
                    nc.vector.tensor_add(l_sb[:], l_sb[:], psl)
                for g in range(ECH):
                    psc = psctxp.tile([128, 128], F32, tag="ctxps",
                                      name=f"cxp{jt}_{g}")
                    nc.tensor.matmul(psc[:, :2 * H],
                                     vc[jt][:, g * 128:(g + 1) * 128],
                                     exm[:], start=True, stop=True)
                    if jt == 0:
                        nc.vector.tensor_copy(ctxu[:, g * 2 * H:(g + 1) * 2 * H],
                                              psc[:, :2 * H])
                    else:
                        nc.vector.tensor_add(ctxu[:, g * 2 * H:(g + 1) * 2 * H],
                                             ctxu[:, g * 2 * H:(g + 1) * 2 * H],
                                             psc[:, :2 * H])

            cc_in = dramp.tile([129, W2], F32)
            cc_out = dramp.tile([NC * 129, W2], F32, addr_space="Shared")
            nc.gpsimd.dma_start(cc_in[0:128, :], ctxu[:])
            nc.gpsimd.dma_start(cc_in[128:129, 0:2 * H], l_sb[:])
            nc.gpsimd.collective_compute(
                "AllGather", mybir.AluOpType.bypass,
                replica_groups=[list(range(NC))],
                ins=[cc_in[:]], outs=[cc_out[:]])

            # ============ local attention phase ============
            wqT = load_wT("wqT")
            qT = proj_T(wqT, lambda wTc, n0, nn: own_cols(wTc), 2 * TPC,
                        scale=SCALE, tagp="q")
            wkT = load_wT("wkT")
            kT = proj_T(wkT, lambda wTc, n0, nn: wTc[:, n0:n0 + nn], 2 * HPAD,
                        tagp="k")
            wvT = load_wT("wvT")
            v = proj_tok(wvT, lambda hc, t: hc[:, t * 128:(t + 1) * 128],
                         2 * HPAD, tagp="l")

            ctxT = [actsp.tile([128, 2 * TPC], BF16, tag=f"ctxT{c}",
                               name=f"ctxT{c}") for c in range(ECH)]
            for b in range(B):
                for qt in range(NQT):
                    vt0 = b * (HPAD // 128) + qt
                    qcol = b * TPC + qt * 128
                    kcol = b * HPAD + qt * 128
                    for h_ in range(H):
                        ti = (b * NQT + qt) * H + h_
                        d0 = h_ * D
                        ct, par = h_ // 2, (h_ % 2) * D
                        kc_, kp = d0 // 128, d0 % 128
                        spool, stag = ((psattn, "attnps") if ti % 2 == 0
                                       else (psproj, "projps"))
                        ps = spool.tile([128, 512], F32, tag=stag,
                                        name=f"aps{ti}")
                        sc = ps[:, 0:KW]
                        nc.tensor.matmul(sc, qT[kc_][kp:kp + D, qcol:qcol + 128],
                                         kT[kc_][kp:kp + D, kcol:kcol + KW],
                                         start=True, stop=True)
                        msk = attnp.tile([128, KW], BF16, tag="msk",
                                         name=f"msk{ti}")
                        nc.vector.tensor_add(msk[:], sc,
                                             mask_q[:, qt * KW:(qt + 1) * KW])
                        att = attnp.tile([128, KW], BF16, tag="att",
                                         name=f"att{ti}")
                        l = attnp.tile([128, 1], F32, tag="l", name=f"l{ti}")
                        nc.scalar.activation(att[:], msk[:], AF.Exp,
                                             accum_out=l[:])
                        lrec = attnp.tile([128, 1], F32, tag="lrec",
                                          name=f"lr{ti}")
                        nc.vector.reciprocal(lrec[:], l[:])
                        attn = attnp.tile([128, KW], BF16, tag="attn",
                                          name=f"at{ti}")
                        nc.vector.tensor_scalar_mul(attn[:], att[:], lrec[:])
                        pt1 = pstp.tile([128, 128], BF16, tag="tps",
                                        name=f"p1{ti}")
                        nc.tensor.transpose(pt1[:], attn[:, 0:128], eye[:])
                        pt2 = pstp.tile([128, 128], BF16, tag="tps",
                                        name=f"p2{ti}")
                        nc.tensor.transpose(pt2[0:32, :],
                                            attn[:, 128:KW], eye[:])
                        aT1 = attnp.tile([128, 128], BF16, tag="aT1",
                                         name=f"a1{ti}")
                        aT2 = attnp.tile([32, 32], BF16, tag="aT2",
                                         name=f"a2{ti}")
                        nc.vector.tensor_copy(aT1[:], pt1[:])
                        nc.vector.tensor_copy(aT2[:], pt2[0:32, 96:128])
                        psc = psctxp.tile([128, 128], F32, tag="ctxps",
                                          name=f"pc{ti}")
                        nc.tensor.matmul(psc[0:64, :], v[vt0][:, d0:d0 + D],
                                         aT1[:], start=True, stop=False,
                                         skip_group_check=True)
                        nc.tensor.matmul(psc[0:64, 96:128],
                                         v[vt0 + 1][0:32, d0:d0 + D],
                                         aT2[:], start=False, stop=True,
                                         skip_group_check=True)
                        nc.scalar.copy(ctxT[ct][par:par + D, qcol:qcol + 128],
                                       psc[0:64, :])

            # ============ cls combine ============
            cc_sb = clsp.tile([128, NC * W2], F32, tag="ccsb")
            nc.sync.dma_start(
                cc_sb[:].rearrange("p (n f) -> p n f", n=NC),
                cc_out[:].rearrange("(n p) f -> p n f", p=129)[0:128])
            cl_sb = clsp.tile([H, NC * B], F32, tag="clsb")
            nc.sync.dma_start(
                cl_sb[:].rearrange("h (n b) -> h n b", n=NC),
                cc_out[:].rearrange("(n p) f -> p n f", p=129)[128:129, :, 0:2 * H]
                .rearrange("o n (h b) -> h n (o b)", b=B))
            csum = clsp.tile([128, W2], F32, tag="csum")
            cview = cc_sb[:].rearrange("p (n f) -> p n f", n=NC)
            nc.vector.tensor_add(csum[:], cview[:, 0, :], cview[:, 1, :])
            for n in range(2, NC):
                nc.vector.tensor_add(csum[:], csum[:], cview[:, n, :])
            lsum = clsp.tile([H, B], F32, tag="lsum")
            lview = cl_sb[:].rearrange("h (n b) -> h n b", n=NC)
            nc.vector.tensor_add(lsum[:], lview[:, 0, :], lview[:, 1, :])
            for n in range(2, NC):
                nc.vector.tensor_add(lsum[:], lsum[:], lview[:, n, :])
            lrec = clsp.tile([H, B], F32, tag="lrecc")
            nc.vector.reciprocal(lrec[:], lsum[:])
            lrec_bf = clsp.tile([H, B], BF16, tag="lrecb")
            nc.vector.tensor_copy(lrec_bf[:], lrec[:])
            ctxcN = clsp.tile([128, ECH * B], BF16, tag="ctxcN")
            for g in range(ECH):
                lrb = psctxp.tile([128, 128], F32, tag="ctxps")
                nc.tensor.matmul(lrb[:, 0:B], ehot[:, g * 128:(g + 1) * 128],
                                 lrec_bf[:], start=True, stop=True)
                sel = clsp.tile([128, B], F32, tag="sel")
                for half in range(2):
                    h_ = 2 * g + half
                    nc.vector.tensor_copy(
                        sel[half * 64:(half + 1) * 64, :],
                        csum[half * 64:(half + 1) * 64,
                             g * 2 * H + 2 * h_:g * 2 * H + 2 * h_ + B])
                nc.vector.tensor_mul(ctxcN[:, g * B:(g + 1) * B], sel[:],
                                     lrb[:, 0:B])

            # ============ output projections ============
            woT = load_wT("woT")
            for t in range(2 * TPC // 128):
                ost = outstp.tile([128, E], F32, tag="ost", name=f"ost{t}")
                for n0 in range(0, E, 512):
                    ps = psproj.tile([128, 512], F32, tag="projps",
                                     name=f"ppo{t}_{n0}")
                    for c in range(ECH):
                        nc.tensor.matmul(ps[:],
                                         ctxT[c][:, t * 128:(t + 1) * 128],
                                         woT[c][:, n0:n0 + 512],
                                         start=(c == 0), stop=(c == ECH - 1))
                    nc.vector.tensor_copy(ost[:, n0:n0 + 512], ps[:])
                nc.sync.dma_start(out_ext[t * 128:(t + 1) * 128, :], ost[:])
            # ============ cls output projection ============
            wocT = load_wT("wocT")
            ost0 = outstp.tile([B, E], F32, tag="ost0")
            for n0 in range(0, E, 512):
                ps = psproj.tile([128, 512], F32, tag="projps")
                for c in range(ECH):
                    nc.tensor.matmul(ps[0:B, :], ctxcN[:, c * B:(c + 1) * B],
                                     wocT[c][:, n0:n0 + 512],
                                     start=(c == 0), stop=(c == ECH - 1))
                nc.vector.tensor_copy(ost0[:, n0:n0 + 512], ps[0:B, :])
            nc.sync.dma_start(out_ext[2 * TPC:2 * TPC + B, :], ost0[:])

    nc.compile()
    return nc


def _bf16(a):
    import ml_dtypes
    return np.ascontiguousarray(np.asarray(a, np.float32)).astype(ml_dtypes.bfloat16)


def make_shards(hidden_states, in_w_cls, out_w_cls, in_w_loc, out_w_loc):
    hs = np.asarray(hidden_states, np.float32)
    iwl = np.asarray(in_w_loc, np.float32)
    iwc = np.asarray(in_w_cls, np.float32)
    weights = {
        "wqT": _bf16(iwl[0:E].T), "wkT": _bf16(iwl[E:2 * E].T),
        "wvT": _bf16(iwl[2 * E:3 * E].T),
        "wqcT": _bf16(iwc[0:E].T), "wkcT": _bf16(iwc[E:2 * E].T),
        "wvcT": _bf16(iwc[2 * E:3 * E].T),
        "woT": _bf16(np.asarray(out_w_loc, np.float32).T),
        "wocT": _bf16(np.asarray(out_w_cls, np.float32).T),
    }
    eye = _bf16(np.eye(128, dtype=np.float32))
    ehot = np.zeros((H, E), np.float32)
    for h_ in range(H):
        ehot[h_, h_ * D:(h_ + 1) * D] = 1.0
    ehot = _bf16(ehot)
    mcls = np.zeros((128, 2 * 2 * H), np.float32)
    for bb in range(2):
        for h_ in range(H):
            mcls[:, bb * 2 * H + 2 * h_ + bb] = 1.0
    mcls = _bf16(mcls)
    # h0T packed [128, ECH*B]: h0T[p, c*B+b] = hs[b, 0, c*128+p]
    h0 = hs[:, 0, :]                         # [B, E]
    h0T = np.transpose(h0.reshape(B, ECH, 128), (2, 1, 0)).reshape(128, ECH * B)
    h0T = _bf16(h0T)
    jr = np.arange(KW)[:, None]
    pr = np.arange(128)[None, :]
    band = (jr - pr >= 0) & (jr - pr <= 2 * WIN)        # [KW, 128] keys x q
    ins = []
    for i in range(NC):
        t0 = i * TPC
        h = np.zeros((2 * HPAD, E), np.float32)
        for b in range(B):
            lo, hi = t0 - WIN, t0 + TPC + WIN
            slo, shi = max(lo, 0), min(hi, S)
            h[b * HPAD + (slo - lo):b * HPAD + (slo - lo) + (shi - slo)] = \
                hs[b, slo:shi]
        hT = _bf16(h.T)                                  # [E, 2*HPAD]
        mask = np.full((128, NQT * KW), NEG, np.float32)  # q-major additive
        for qt in range(NQT):
            tk = t0 - WIN + qt * 128 + np.arange(KW)     # global key token
            kvalid = (tk >= 1) & (tk < S)
            valid = (band & kvalid[:, None]).T           # [128 q, KW]
            mask[:, qt * KW:(qt + 1) * KW] = np.where(valid, 0.0, NEG)
        ins.append(dict(hT=hT, h0T=h0T, mask_q=_bf16(mask),
                        mask_cls=mcls, ehot=ehot, eye=eye, **weights))
    return ins


_NC_CACHE = None


def kernel(hidden_states, in_w_cls, in_b_cls, out_w_cls, out_b_cls,
           in_w_loc, in_b_loc, out_w_loc, out_b_loc, _want_results=False,
           _trace=False, _tmpdir=None):
    global _NC_CACHE
    if _NC_CACHE is None:
        _NC_CACHE = build_kernel()
    ins = make_shards(hidden_states, in_w_cls, out_w_cls, in_w_loc, out_w_loc)
    try:
        res = run_bass_kernel_spmd(_NC_CACHE, ins, core_ids=list(range(NC)),
                                   trace=_trace, tmpdir=_tmpdir)
    except Exception:
        # transient device errors (e.g. NRT_EXEC_UNIT_UNRECOVERABLE from a
        # prior wedged run) clear on re-execution; retry once
        res = run_bass_kernel_spmd(_NC_CACHE, ins, core_ids=list(range(NC)),
                                   trace=_trace, tmpdir=_tmpdir)
    outs = [np.asarray(r["out"]) for r in res.results]
    full = np.zeros((B, S, E), np.float32)
    for i in range(NC):
        for b in range(B):
            full[b, i * TPC:(i + 1) * TPC] = outs[i][b * TPC:(b + 1) * TPC]
    full[:, 0, :] = outs[0][2 * TPC:2 * TPC + B]
    full[:, 0, :] += np.asarray(out_b_cls, np.float32)
    full[:, 1:, :] += np.asarray(out_b_loc, np.float32)
    if _want_results:
        return full, res
    return full

